# revision 16
# baseline (speedup 1.0000x reference)
"""Trainium2 Bass kernel for nn_HardwareOptimizedSpikeProcessor.

Reference semantics (per timestep t):
    acc += (s_t @ (W*mask).T) * 2**scale_exp     # [B, Cout]
    spk  = acc >= 2**threshold_exp
    acc  = acc * (1 - spk)
    out[:, :, t] = spk

Strategy (v2):
  - Shard batch/2 x cout/4: each of the 8 cores handles 32 samples x 512
    output channels.  PE work per core is unchanged (8.6 GFLOP bf16), but the
    matmul free dim is b*tb = 32*8 = 256 at a t-block of only 8 steps, which
    keeps LDWEIGHTS fully amortized while letting the sequential scan pipeline
    against the PE at fine (8-step) granularity -- the scan tail after the
    last matmul shrinks from ~60us (baseline) to ~6us.
  - The matmul contribution c[t] = s_t @ Wm.T is exact in bf16 (spikes are
    0/1; masked weights are ints in [-127,127]); PSUM accumulates fp32.
  - Scan step is 2 DVE instructions instead of 3:
        u_t  = acc + c_t                         (tensor_tensor add)
        acc  = (u_t < thr) * u_t                 (scalar_tensor_tensor)
    and spikes (u_t >= thr) are extracted in bulk per 8-step block on the
    otherwise-idle Pool engine, off the serial chain.
  - PE p-state warm-up: ~3us of junk matmuls issued while the first DMAs
    land, so real matmuls run at full clock from the start.
"""

import sys

for _p in ("/opt/trn_rl_repo",):
    if _p not in sys.path:
        sys.path.insert(0, _p)

import numpy as np
import ml_dtypes

import concourse.bass as bass
import concourse.mybir as mybir
import concourse.tile as tile
from concourse.bass_utils import run_bass_kernel_spmd

B, CIN, COUT, T = 64, 2048, 2048, 128
NCORES = 8
NB = 2                      # batch shards
NQ = 4                      # cout shards
BLOC = B // NB              # 32 samples per core
QLOC = COUT // NQ           # 512 output channels per core
MC = QLOC // 128            # 4 output-channel chunks per core
KC = CIN // 128             # 16 contraction chunks


# scan blocks along T: small at the head (PE can start on a small first
# spike chunk) and at the tail (short post-matmul scan); 16-step blocks in
# the middle where the 512-wide matmul free dim gives the best PE cadence
# (LDWEIGHTS fully hidden: 213ns/mm vs 2x109.3 at 256-wide).
BLOCKS = [(0, 8), (8, 8)] + [(16 * i, 16) for i in range(1, 7)] + [
    (112, 8), (120, 8)
]
TBMAX = 16
# spike DMA chunks along T (first small so PE starts early; every block
# falls entirely within one chunk)
TCHUNKS = [8, 8, 32, 32, 32, 16]
NBLK = len(BLOCKS)
assert sum(TCHUNKS) == T
assert sum(tb for _, tb in BLOCKS) == T

_MAX_WAITS = 1


def _split_excess_waits(nc):
    """This container's walrus build accepts at most one sync-wait per
    instruction; spill extra waits onto same-engine NOPs placed before the
    offending instruction."""
    for f in nc.m.functions:
        for bb in f.blocks:
            new_list = []
            for ins in bb.instructions:
                si = ins.sync_info
                waits = list(si.on_wait) if si is not None and si.on_wait else []
                if len(waits) > _MAX_WAITS:
                    extra, keep = waits[:-_MAX_WAITS], waits[-_MAX_WAITS:]
                    for i in range(0, len(extra), _MAX_WAITS):
                        nop = mybir.InstNoOp(
                            name=f"{ins.name}-waitsplit-{i}", ins=[], outs=[]
                        )
                        nop.engine = ins.engine
                        nop.sync_info = mybir.SyncInfo(
                            on_wait=extra[i : i + _MAX_WAITS], on_update=[]
                        )
                        new_list.append(nop)
                    ins.sync_info = mybir.SyncInfo(
                        on_wait=keep,
                        on_update=list(si.on_update) if si.on_update else [],
                    )
                new_list.append(ins)
            bb.instructions[:] = new_list


def _build(thr: float):
    f32 = mybir.dt.float32
    bf16 = mybir.dt.bfloat16
    fp8 = mybir.dt.float8e4
    u8 = mybir.dt.uint8
    nc = bass.Bass()

    # W^T (2**scale_exp folded in): per m-chunk [cin_lo, k, cout_lo]
    wt_ds = [
        nc.dram_tensor(f"wt{m}", [128, KC, 128], bf16, kind="ExternalInput")
        for m in range(MC)
    ]
    # spike chunks, each contiguous [cin_lo, k, b, tc]
    spk_ds = [
        nc.dram_tensor(f"spk{j}", [128, KC, BLOC, tc], fp8, kind="ExternalInput")
        for j, tc in enumerate(TCHUNKS)
    ]
    # per-block spike outputs [cout_lo, t, m, b]
    out_ds = [
        nc.dram_tensor(f"out{j}", [128, tb, MC, BLOC], u8, kind="ExternalOutput")
        for j, (_, tb) in enumerate(BLOCKS)
    ]

    # block -> (chunk index, t offset within chunk)
    cstart = []
    s = 0
    for tc in TCHUNKS:
        cstart.append(s)
        s += tc
    blk_map = []
    for t0, tb in BLOCKS:
        cj = max(i for i, cs in enumerate(cstart) if cs <= t0)
        assert t0 + tb <= cstart[cj] + TCHUNKS[cj]
        blk_map.append((cj, t0 - cstart[cj]))

    with tile.TileContext(nc) as tc:
        with (
            tc.tile_pool(name="const", bufs=1) as const,
            tc.tile_pool(name="cpool", bufs=3) as cpool,
            tc.tile_pool(name="upool", bufs=3) as upool,
            tc.tile_pool(name="opool", bufs=3) as opool,
            tc.tile_pool(name="psum", bufs=2, space="PSUM") as psum,
        ):
            wt_sb = const.tile([128, MC, KC, 128], bf16)
            spk_sbs = [
                const.tile([128, KC, BLOC, tc], fp8, name=f"spk_sb{j}")
                for j, tc in enumerate(TCHUNKS)
            ]
            acc = const.tile([128, MC, BLOC], f32)
            junk = const.tile([128, 256], bf16)

            nc.vector.memset(acc[:], 0.0)
            nc.vector.memset(junk[:], 0.0)

            # DMA order: first spike chunk + weights first so PE starts ASAP
            nc.sync.dma_start(spk_sbs[0][:], spk_ds[0][:])
            for m in range(MC):
                nc.sync.dma_start(wt_sb[:, m], wt_ds[m][:])
            for j in range(1, len(TCHUNKS)):
                nc.sync.dma_start(spk_sbs[j][:], spk_ds[j][:])

            # PE p-state warm-up on junk data while the DMAs land; the warm-up
            # psum borrows a slot of the main psum ring (freed before block 1
            # needs it)
            wps = psum.tile([128, MC, BLOC * TBMAX], f32, tag="ps", name="ps")
            for _ in range(23):
                nc.tensor.matmul(wps[:, 0, :256], lhsT=junk[:, :128], rhs=junk[:])

            for j, (t0, tb) in enumerate(BLOCKS):
                cj, toff = blk_map[j]
                nfree = BLOC * tb
                ps = psum.tile([128, MC, BLOC * TBMAX], f32, tag="ps", name="ps")
                for m in range(MC):
                    for k in range(KC):
                        nc.tensor.matmul(
                            ps[:, m, :nfree],
                            lhsT=wt_sb[:, m, k, :],
                            rhs=spk_sbs[cj][:, k, :, toff : toff + tb],
                            start=(k == 0),
                            stop=(k == KC - 1),
                        )
                # PSUM [p, m, (b t)] -> SBUF c [p, t, m, b] so each scan step
                # reads a contiguous [128, (m b)] slice.  Drained in 8-step
                # granules so the scan starts while later granules drain.
                last = j == NBLK - 1
                ps_v = ps[:, :, :nfree].rearrange("p m (b t) -> p m b t", b=BLOC)
                H = 4 if tb == 8 else 8
                ngr = tb // H
                if not last:
                    c = cpool.tile([128, TBMAX, MC, BLOC], f32, tag="cblk")
                    for h in range(ngr):
                        nc.scalar.copy(
                            c[:, h * H : (h + 1) * H].rearrange(
                                "p t m b -> p m b t"
                            ),
                            ps_v[:, :, :, h * H : (h + 1) * H],
                        )
                u = upool.tile([128, TBMAX, MC, BLOC], f32, tag="ublk")
                ob = opool.tile([128, TBMAX, MC, BLOC], u8, tag="oblk")
                for h in range(ngr):
                    for t in range(h * H, (h + 1) * H):
                        # last block: read c straight from PSUM (skips the
                        # ACT drain latency right at the kernel tail)
                        c_t = ps_v[:, :, :, t] if last else c[:, t]
                        nc.vector.tensor_tensor(
                            u[:, t], acc[:], c_t, mybir.AluOpType.add
                        )
                        nc.vector.scalar_tensor_tensor(
                            acc[:], u[:, t], thr, u[:, t],
                            mybir.AluOpType.is_lt, mybir.AluOpType.mult,
                        )
                    # spikes = (u >= thr), per granule off the serial chain
                    # (DVE: the Pool engine runs tensor_scalar ~10x slower
                    # than its nominal rate, measured 15.6us per block)
                    nc.vector.tensor_scalar(
                        ob[:, h * H : (h + 1) * H],
                        u[:, h * H : (h + 1) * H],
                        thr, None, mybir.AluOpType.is_ge,
                    )
                    nc.sync.dma_start(
                        out_ds[j][:, h * H : (h + 1) * H],
                        ob[:, h * H : (h + 1) * H],
                    )

    _split_excess_waits(nc)
    return nc


def _prep_inputs(spikes, weights, mask, scale_exp):
    wm = weights * mask  # integers <= 127, exact
    scale = np.exp2(scale_exp.astype(np.float64)).astype(np.float32)
    wm = (wm * scale[:, None]).astype(np.float32)  # fold power-of-2 scale in
    in_maps = []
    for core in range(NCORES):
        bh, cq = divmod(core, NQ)
        # weights for this cout shard: [qloc, cin] -> W^T -> [m, cin_lo, k, cout_lo]
        wq = wm[cq * QLOC : (cq + 1) * QLOC]  # [512, 2048]
        wt = (
            wq.T.reshape(KC, 128, MC, 128)
            .transpose(2, 1, 0, 3)
            .astype(ml_dtypes.bfloat16)
        )  # [m, cin_lo, k, cout_lo]
        m = {f"wt{mm}": np.ascontiguousarray(wt[mm]) for mm in range(MC)}
        # spikes for this batch shard: [b, cin, t] -> [cin_lo, k, b, t]
        s = spikes[bh * BLOC : (bh + 1) * BLOC]
        a = s.transpose(1, 0, 2).reshape(KC, 128, BLOC, T).transpose(1, 0, 2, 3)
        a = a.astype(ml_dtypes.float8_e4m3)
        t0 = 0
        for jj, tc in enumerate(TCHUNKS):
            m[f"spk{jj}"] = np.ascontiguousarray(a[:, :, :, t0 : t0 + tc])
            t0 += tc
        in_maps.append(m)
    return in_maps


_CACHE = {}


def _get_program(thr: float):
    if thr not in _CACHE:
        _CACHE[thr] = _build(thr)
    return _CACHE[thr]


def kernel(spikes, weights, mask, scale_exp, threshold_exp, **run_kwargs):
    thr = float(2.0 ** int(np.asarray(threshold_exp)))
    nc = _get_program(thr)
    in_maps = _prep_inputs(
        np.asarray(spikes, dtype=np.float32),
        np.asarray(weights, dtype=np.float32),
        np.asarray(mask, dtype=np.float32),
        np.asarray(scale_exp),
    )
    res = run_bass_kernel_spmd(
        nc, in_maps, core_ids=list(range(NCORES)), **run_kwargs
    )
    full = np.zeros((B, COUT, T), dtype=np.float32)
    for core in range(NCORES):
        bh, cq = divmod(core, NQ)
        blks = [
            np.asarray(res.results[core][f"out{j}"]) for j in range(NBLK)
        ]  # each [cout_lo, t, m, b]
        a = np.concatenate(blks, axis=1)  # [cout_lo, T, m, b]
        # -> [b, m, cout_lo, T] -> [b_loc, qloc, T]
        a = a.transpose(3, 2, 0, 1).reshape(BLOC, QLOC, T)
        full[bh * BLOC : (bh + 1) * BLOC, cq * QLOC : (cq + 1) * QLOC] = a
    if run_kwargs:
        return full, res
    return full
